# revision 12
# baseline (speedup 1.0000x reference)
"""Causal GQA self-attention block (B=2, T=2048, C=2048, 16 q-heads / 4 kv-heads,
head_dim=128, RoPE + RMS-norm on q/k) for 8 Trainium2 NeuronCores.

Sharding: core = (batch b, kv-group g), b in {0,1}, g in {0..3}.
Each core computes its batch's projections for 4 q-heads + 1 kv head,
causal attention, and a partial output projection (Wo row-shard).
Host sums the 4 partials per batch.
"""

import sys

for _p in ("/opt/trn_rl_repo", "/root/.axon_site/_ro/trn_rl_repo"):
    if _p not in sys.path:
        sys.path.insert(0, _p)

import os

import numpy as np

import concourse.bass as bass  # noqa: F401
import concourse.mybir as mybir
from concourse import bacc
from concourse.tile import TileContext
from concourse.bass_utils import run_bass_kernel_spmd

P = 128
T = 2048
C = 2048
KT = C // P          # 16 contraction tiles
TB = T // P          # 16 T1 blocks
NH = 4               # q heads per core
D = 128              # head dim
SCALE = 1.0 / np.sqrt(D)
EPS = float(np.finfo(np.float32).eps)
NEG = -1.0e30

F32R = mybir.dt.float32r
F32 = mybir.dt.float32
AF = mybir.ActivationFunctionType
ALU = mybir.AluOpType

_NC_CACHE = None


def build_nc():
    nc = bacc.Bacc("TRN2", target_bir_lowering=False, debug=False)

    xt = nc.dram_tensor("xt", [C, T], F32R, kind="ExternalInput")
    wq = nc.dram_tensor("wq", [C, NH * D], F32R, kind="ExternalInput")
    wkv = nc.dram_tensor("wkv", [C, 2 * D], F32R, kind="ExternalInput")
    wo = nc.dram_tensor("wo", [NH * D, C], F32R, kind="ExternalInput")
    csa = nc.dram_tensor("csa", [T, 128], F32, kind="ExternalInput")
    csb = nc.dram_tensor("csb", [T, 128], F32, kind="ExternalInput")
    diag = nc.dram_tensor("diag", [P, P], F32, kind="ExternalInput")
    ident = nc.dram_tensor("ident", [P, P], F32R, kind="ExternalInput")
    onesc = nc.dram_tensor("onesc", [P, 1], F32R, kind="ExternalInput")
    onesr = nc.dram_tensor("onesr", [1, P], F32R, kind="ExternalInput")
    epsc = nc.dram_tensor("epsc", [P, 1], F32, kind="ExternalInput")
    y = nc.dram_tensor("y", [T, C], F32, kind="ExternalOutput")

    xt_v = xt.ap().rearrange("(co ci) t -> ci co t", ci=P)
    wq_v = wq.ap().rearrange("(co ci) n -> ci co n", ci=P)
    wkv_v = wkv.ap().rearrange("(co ci) n -> ci co n", ci=P)
    wo_v = wo.ap().rearrange("(h d) n -> d h n", d=P)

    with TileContext(nc) as tc:
        with tc.tile_pool(name="resident", bufs=1) as wpool:
            wq_sb = wpool.tile([P, KT, NH * D], F32R)
            wkv_sb = wpool.tile([P, KT, 2 * D], F32R)
            wo_sb = wpool.tile([P, NH, C], F32R)
            kt_sb = wpool.tile([P, T], F32R)
            v_sb = wpool.tile([P, TB, D], F32R)
            diag_sb = wpool.tile([P, P], F32)
            ident_sb = wpool.tile([P, P], F32R)
            onesc_sb = wpool.tile([P, 1], F32R)
            onesr_sb = wpool.tile([1, P], F32R)
            eps_sb = wpool.tile([P, 1], F32)

            # per-k-tile weight loads so the first matmuls start early
            for ki in range(KT):
                nc.sync.dma_start(out=wq_sb[:, ki, :], in_=wq_v[:, ki, :])
                nc.sync.dma_start(out=wkv_sb[:, ki, :], in_=wkv_v[:, ki, :])
            nc.sync.dma_start(out=diag_sb[:], in_=diag.ap())
            nc.sync.dma_start(out=ident_sb[:], in_=ident.ap())
            nc.sync.dma_start(out=onesc_sb[:], in_=onesc.ap())
            nc.sync.dma_start(out=onesr_sb[:], in_=onesr.ap())
            nc.sync.dma_start(out=eps_sb[:], in_=epsc.ap())

            for h2 in range(2):  # T1 halves
                with tc.tile_pool(name="halfbuf", bufs=1) as halfpool:
                    qt_sb = halfpool.tile([P, NH, T // 2], F32R)
                    ot_sb = halfpool.tile([P, NH, T // 2], F32R)

                    # ---- phase 1: projections + rope + rms + transposes ----
                    with tc.tile_pool(name="p1sb", bufs=2) as p1sb, \
                         tc.tile_pool(name="p1small", bufs=2) as p1small, \
                         tc.tile_pool(name="p1q", bufs=2, space="PSUM") as p1q, \
                         tc.tile_pool(name="p1kv", bufs=2, space="PSUM") as p1kv, \
                         tc.tile_pool(name="p1qt", bufs=2, space="PSUM") as p1qt, \
                         tc.tile_pool(name="p1kt", bufs=1, space="PSUM") as p1kt:
                        for tbl in range(TB // 2):
                            tb = h2 * (TB // 2) + tbl
                            xts = p1sb.tile([P, KT, P], F32R, tag="xts")
                            nc.sync.dma_start(
                                out=xts[:], in_=xt_v[:, :, tb * P:(tb + 1) * P])
                            # cos/sin rows for this block, replicated 4x (free-dim
                            # broadcast read) so rope runs as whole-tile ops
                            ca4 = p1sb.tile([P, NH, 128], F32, tag="ca4")
                            cb4 = p1sb.tile([P, NH, 128], F32, tag="cb4")
                            nc.sync.dma_start(
                                out=ca4[:],
                                in_=csa.ap()[tb * P:(tb + 1) * P, :].unsqueeze(1)
                                .broadcast_to([P, NH, 128]))
                            nc.sync.dma_start(
                                out=cb4[:],
                                in_=csb.ap()[tb * P:(tb + 1) * P, :].unsqueeze(1)
                                .broadcast_to([P, NH, 128]))

                            q_ps = p1q.tile([P, NH * D], F32)
                            kv_ps = p1kv.tile([P, 2 * D], F32)
                            for ki in range(KT):
                                nc.tensor.matmul(q_ps[:], xts[:, ki, :], wq_sb[:, ki, :],
                                                 start=(ki == 0), stop=(ki == KT - 1))
                            for ki in range(KT):
                                nc.tensor.matmul(kv_ps[:], xts[:, ki, :], wkv_sb[:, ki, :],
                                                 start=(ki == 0), stop=(ki == KT - 1))

                            # v pass-through
                            nc.vector.tensor_copy(out=v_sb[:, tb, :], in_=kv_ps[:, D:2 * D])

                            # rope: csa = [cos|sin], csb = [sin|cos]
                            q4 = q_ps[:].rearrange("p (h d) -> p h d", h=NH)
                            pa = p1sb.tile([P, NH, D], F32, tag="pa")
                            pb = p1sb.tile([P, NH, D], F32, tag="pb")
                            pka = p1sb.tile([P, D], F32, tag="pka")
                            pkb = p1sb.tile([P, D], F32, tag="pkb")
                            nc.vector.tensor_mul(pa[:], q4, ca4[:])
                            nc.vector.tensor_mul(pb[:], q4, cb4[:])
                            nc.vector.tensor_mul(pka[:], kv_ps[:, 0:D], ca4[:, 0, :])
                            nc.vector.tensor_mul(pkb[:], kv_ps[:, 0:D], cb4[:, 0, :])

                            rq = p1sb.tile([P, NH, D], F32, tag="rq")
                            rk = p1sb.tile([P, D], F32, tag="rk")
                            nc.vector.tensor_add(rq[:, :, 0:64], pa[:, :, 0:64], pa[:, :, 64:128])
                            nc.vector.tensor_sub(rq[:, :, 64:128], pb[:, :, 64:128], pb[:, :, 0:64])
                            nc.vector.tensor_add(rk[:, 0:64], pka[:, 0:64], pka[:, 64:128])
                            nc.vector.tensor_sub(rk[:, 64:128], pkb[:, 64:128], pkb[:, 0:64])

                            # rms-norm factors (sum of squares via stt accum)
                            sqs = p1sb.tile([P, NH, D], F32, tag="sqs")
                            sqk = p1sb.tile([P, D], F32, tag="sqk")
                            ss = p1small.tile([P, 8], F32, tag="ss")
                            for h in range(NH):
                                nc.vector.scalar_tensor_tensor(
                                    out=sqs[:, h, :], in0=rq[:, h, :], scalar=1.0,
                                    in1=rq[:, h, :], op0=ALU.mult, op1=ALU.mult,
                                    accum_out=ss[:, h:h + 1])
                            nc.vector.scalar_tensor_tensor(
                                out=sqk[:], in0=rk[:], scalar=1.0,
                                in1=rk[:], op0=ALU.mult, op1=ALU.mult,
                                accum_out=ss[:, NH:NH + 1])
                            sq = p1small.tile([P, 8], F32, tag="sq")
                            nc.scalar.activation(sq[:, 0:NH + 1], ss[:, 0:NH + 1], AF.Sqrt,
                                                 bias=eps_sb[:], scale=1.0 / D)
                            rs = p1small.tile([P, 8], F32, tag="rs")
                            nc.vector.reciprocal(rs[:, 0:NH + 1], sq[:, 0:NH + 1])

                            qn = p1sb.tile([P, NH, D], F32R, tag="qn")
                            kn = p1sb.tile([P, D], F32R, tag="kn")
                            for h in range(NH):
                                nc.vector.tensor_scalar_mul(qn[:, h, :], rq[:, h, :], rs[:, h:h + 1])
                            nc.vector.tensor_scalar_mul(kn[:], rk[:], rs[:, NH:NH + 1])

                            # transposes -> [D, T1] layout
                            qt_ps = p1qt.tile([P, NH * D], F32R)
                            for h in range(NH):
                                nc.tensor.transpose(qt_ps[:, h * D:(h + 1) * D], qn[:, h, :], ident_sb[:])
                            kt_ps = p1kt.tile([P, D], F32R)
                            nc.tensor.transpose(kt_ps[:], kn[:], ident_sb[:])

                            nc.vector.tensor_copy(
                                out=qt_sb[:, :, tbl * P:(tbl + 1) * P],
                                in_=qt_ps[:].rearrange("p (h t) -> p h t", h=NH))
                            nc.vector.tensor_copy(out=kt_sb[:, tb * P:(tb + 1) * P], in_=kt_ps[:])

                    if h2 == 0:
                        # wo only needed from the first out-projection; loading
                        # here keeps it off the critical startup DMA path
                        for h in range(NH):
                            nc.sync.dma_start(out=wo_sb[:, h, :], in_=wo_v[:, h, :])

                    # ---- phase 2+3: attention (S^T layout) + out-projection ----
                    with tc.tile_pool(name="p2pt", bufs=4) as p2pt, \
                         tc.tile_pool(name="p2small", bufs=2) as p2small, \
                         tc.tile_pool(name="p3sb", bufs=3) as p3sb, \
                         tc.tile_pool(name="p2s", bufs=3, space="PSUM") as p2s, \
                         tc.tile_pool(name="p2o", bufs=2, space="PSUM") as p2o, \
                         tc.tile_pool(name="p2l", bufs=1, space="PSUM") as p2l, \
                         tc.tile_pool(name="p2rb", bufs=1, space="PSUM") as p2rb, \
                         tc.tile_pool(name="p3y", bufs=1, space="PSUM") as p3y:
                        for c2l in range(2):
                            c2 = 2 * h2 + c2l
                            for h in range(NH):
                                o_ps = p2o.tile([P, 512], F32)
                                l_ps = p2l.tile([1, 512], F32)
                                m_last = 4 * c2 + 3
                                for m in range(4 * c2 + 4):
                                    dg = m - 4 * c2
                                    n0 = 128 * dg if dg > 0 else 0
                                    w = 512 - n0
                                    s_ps = p2s.tile([P, 512], F32)
                                    nc.tensor.matmul(
                                        s_ps[:, 0:w],
                                        kt_sb[:, m * P:(m + 1) * P],
                                        qt_sb[:, h, c2l * 512 + n0: (c2l + 1) * 512],
                                        start=True, stop=True)
                                    if dg >= 0:
                                        nc.vector.tensor_add(s_ps[:, 0:P], s_ps[:, 0:P], diag_sb[:])
                                    pt = p2pt.tile([P, 512], F32R)
                                    nc.scalar.activation(pt[:, 0:w], s_ps[:, 0:w], AF.Exp,
                                                         scale=float(SCALE))
                                    nc.tensor.matmul(o_ps[:, n0:512], v_sb[:, m, :], pt[:, 0:w],
                                                     start=(m == 0), stop=(m == m_last),
                                                     skip_group_check=True)
                                    nc.tensor.matmul(l_ps[:, n0:512], onesc_sb[:], pt[:, 0:w],
                                                     start=(m == 0), stop=(m == m_last),
                                                     skip_group_check=True)

                                l_sb = p2small.tile([1, 512], F32, tag="l_sb")
                                nc.vector.tensor_copy(out=l_sb[:], in_=l_ps[:])
                                rl = p2small.tile([1, 512], F32R, tag="rl")
                                with nc.allow_low_precision(reason="f32r softmax recip for PE broadcast"):
                                    nc.vector.reciprocal(rl[:], l_sb[:])
                                rb_ps = p2rb.tile([P, 512], F32)
                                nc.tensor.matmul(rb_ps[:], onesr_sb[:], rl[:], start=True, stop=True)
                                rb_sb = p2small.tile([P, 512], F32, tag="rb_sb")
                                nc.vector.tensor_copy(out=rb_sb[:], in_=rb_ps[:])
                                nc.vector.tensor_mul(
                                    ot_sb[:, h, c2l * 512:(c2l + 1) * 512], o_ps[:], rb_sb[:])

                            # out-projection for this 512-wide T1 chunk
                            for tq in range(4):
                                tbl = c2l * 4 + tq
                                tt = h2 * (TB // 2) + tbl
                                for co in range(4):
                                    y_ps = p3y.tile([P, 512], F32)
                                    for h in range(NH):
                                        nc.tensor.matmul(
                                            y_ps[:],
                                            ot_sb[:, h, tbl * P:(tbl + 1) * P],
                                            wo_sb[:, h, co * 512:(co + 1) * 512],
                                            start=(h == 0), stop=(h == NH - 1))
                                    y_sb = p3sb.tile([P, 512], F32)
                                    nc.scalar.copy(out=y_sb[:], in_=y_ps[:])
                                    nc.sync.dma_start(
                                        out=y.ap()[tt * P:(tt + 1) * P, co * 512:(co + 1) * 512],
                                        in_=y_sb[:])

    nc.compile()
    return nc


def make_in_maps(x, cos, sin, Wq, Wk, Wv, Wo):
    """Build per-core input maps (all host-side numpy preprocessing)."""
    cos2 = np.ascontiguousarray(cos.reshape(T, 64)).astype(np.float32)
    sin2 = np.ascontiguousarray(sin.reshape(T, 64)).astype(np.float32)
    csa = np.concatenate([cos2, sin2], axis=1)          # [T, 128]
    csb = np.concatenate([sin2, cos2], axis=1)          # [T, 128]
    pp, yy = np.meshgrid(np.arange(P), np.arange(P), indexing="ij")
    diag = np.where(yy >= pp, 0.0, NEG).astype(np.float32)  # keep t1 >= t2
    ident = np.eye(P, dtype=np.float32)
    onesc = np.ones((P, 1), dtype=np.float32)
    onesr = np.ones((1, P), dtype=np.float32)

    in_maps = []
    for core in range(8):
        b, g = divmod(core, 4)
        in_maps.append({
            "xt": np.ascontiguousarray(x[b].T),
            "wq": np.ascontiguousarray(Wq[:, 512 * g:512 * (g + 1)]),
            "wkv": np.ascontiguousarray(
                np.concatenate([Wk[:, 128 * g:128 * (g + 1)],
                                Wv[:, 128 * g:128 * (g + 1)]], axis=1)),
            "wo": np.ascontiguousarray(Wo[512 * g:512 * (g + 1), :]),
            "csa": csa,
            "csb": csb,
            "diag": diag,
            "ident": ident,
            "onesc": onesc,
            "onesr": onesr,
            "epsc": np.full((P, 1), EPS, dtype=np.float32),
        })
    return in_maps


def kernel(x, cos, sin, Wq, Wk, Wv, Wo):
    global _NC_CACHE
    x = np.asarray(x, dtype=np.float32)
    cos = np.asarray(cos, dtype=np.float32)
    sin = np.asarray(sin, dtype=np.float32)
    Wq = np.asarray(Wq, dtype=np.float32)
    Wk = np.asarray(Wk, dtype=np.float32)
    Wv = np.asarray(Wv, dtype=np.float32)
    Wo = np.asarray(Wo, dtype=np.float32)

    if _NC_CACHE is None:
        _NC_CACHE = build_nc()
    nc = _NC_CACHE

    in_maps = make_in_maps(x, cos, sin, Wq, Wk, Wv, Wo)
    res = run_bass_kernel_spmd(nc, in_maps, core_ids=list(range(8)))

    out = np.zeros((2, T, C), dtype=np.float32)
    for core in range(8):
        b = core // 4
        out[b] += res.results[core]["y"]
    return out


# revision 13
# speedup vs baseline: 1.0218x; 1.0218x over previous
"""Causal GQA self-attention block (B=2, T=2048, C=2048, 16 q-heads / 4 kv-heads,
head_dim=128, RoPE + RMS-norm on q/k) for 8 Trainium2 NeuronCores.

Sharding: core = (batch b, kv-group g), b in {0,1}, g in {0..3}.
Each core computes its batch's projections for 4 q-heads + 1 kv head,
causal attention, and a partial output projection (Wo row-shard).
Host sums the 4 partials per batch.
"""

import sys

for _p in ("/opt/trn_rl_repo", "/root/.axon_site/_ro/trn_rl_repo"):
    if _p not in sys.path:
        sys.path.insert(0, _p)

import os

import numpy as np

import concourse.bass as bass  # noqa: F401
import concourse.mybir as mybir
from concourse import bacc
from concourse.tile import TileContext
from concourse.bass_utils import run_bass_kernel_spmd

P = 128
T = 2048
C = 2048
KT = C // P          # 16 contraction tiles
TB = T // P          # 16 T1 blocks
NH = 4               # q heads per core
D = 128              # head dim
SCALE = 1.0 / np.sqrt(D)
EPS = float(np.finfo(np.float32).eps)
NEG = -1.0e30

F32R = mybir.dt.float32r
F32 = mybir.dt.float32
AF = mybir.ActivationFunctionType
ALU = mybir.AluOpType

_NC_CACHE = None


def build_nc():
    nc = bacc.Bacc("TRN2", target_bir_lowering=False, debug=False)

    xt = nc.dram_tensor("xt", [C, T], F32R, kind="ExternalInput")
    wq = nc.dram_tensor("wq", [C, NH * D], F32R, kind="ExternalInput")
    wkv = nc.dram_tensor("wkv", [C, 2 * D], F32R, kind="ExternalInput")
    wo = nc.dram_tensor("wo", [NH * D, C], F32R, kind="ExternalInput")
    csa = nc.dram_tensor("csa", [T, 128], F32, kind="ExternalInput")
    csb = nc.dram_tensor("csb", [T, 128], F32, kind="ExternalInput")
    diag = nc.dram_tensor("diag", [P, P], F32, kind="ExternalInput")
    ident = nc.dram_tensor("ident", [P, P], F32R, kind="ExternalInput")
    onesc = nc.dram_tensor("onesc", [P, 1], F32R, kind="ExternalInput")
    onesr = nc.dram_tensor("onesr", [1, P], F32R, kind="ExternalInput")
    epsc = nc.dram_tensor("epsc", [P, 1], F32, kind="ExternalInput")
    y = nc.dram_tensor("y", [T, C], F32, kind="ExternalOutput")

    xt_v = xt.ap().rearrange("(co ci) t -> ci co t", ci=P)
    wq_v = wq.ap().rearrange("(co ci) n -> ci co n", ci=P)
    wkv_v = wkv.ap().rearrange("(co ci) n -> ci co n", ci=P)
    wo_v = wo.ap().rearrange("(h d) n -> d h n", d=P)

    with TileContext(nc) as tc:
        with tc.tile_pool(name="resident", bufs=1) as wpool:
            wq_sb = wpool.tile([P, KT, NH * D], F32R)
            wkv_sb = wpool.tile([P, KT, 2 * D], F32R)
            wo_sb = wpool.tile([P, NH, C], F32R)
            kt_sb = wpool.tile([P, T], F32R)
            v_sb = wpool.tile([P, TB, D], F32R)
            diag_sb = wpool.tile([P, P], F32)
            ident_sb = wpool.tile([P, P], F32R)
            onesc_sb = wpool.tile([P, 1], F32R)
            onesr_sb = wpool.tile([1, P], F32R)
            eps_sb = wpool.tile([P, 1], F32)

            # per-k-tile weight loads so the first matmul chain can start
            # as soon as its first slices land
            for ki in range(KT):
                nc.sync.dma_start(out=wq_sb[:, ki, :], in_=wq_v[:, ki, :])
                nc.sync.dma_start(out=wkv_sb[:, ki, :], in_=wkv_v[:, ki, :])
            nc.sync.dma_start(out=diag_sb[:], in_=diag.ap())
            nc.sync.dma_start(out=ident_sb[:], in_=ident.ap())
            nc.sync.dma_start(out=onesc_sb[:], in_=onesc.ap())
            nc.sync.dma_start(out=onesr_sb[:], in_=onesr.ap())
            nc.sync.dma_start(out=eps_sb[:], in_=epsc.ap())

            for h2 in range(2):  # T1 halves
                with tc.tile_pool(name="halfbuf", bufs=1) as halfpool:
                    qt_sb = halfpool.tile([P, NH, T // 2], F32R)
                    ot_sb = halfpool.tile([P, NH, T // 2], F32R)

                    # ---- phase 1: projections + rope + rms + transposes ----
                    with tc.tile_pool(name="p1x", bufs=3) as p1x, \
                         tc.tile_pool(name="p1sb", bufs=2) as p1sb, \
                         tc.tile_pool(name="p1small", bufs=2) as p1small, \
                         tc.tile_pool(name="p1q", bufs=2, space="PSUM") as p1q, \
                         tc.tile_pool(name="p1kv", bufs=2, space="PSUM") as p1kv, \
                         tc.tile_pool(name="p1qt", bufs=2, space="PSUM") as p1qt, \
                         tc.tile_pool(name="p1kt", bufs=1, space="PSUM") as p1kt:
                        for tbl in range(TB // 2):
                            tb = h2 * (TB // 2) + tbl
                            xts = p1x.tile([P, KT, P], F32R, tag="xts")
                            if tb == 0:
                                # interleave with the weight loads in consumption
                                # order so the first matmul chain starts early
                                for ki in range(KT):
                                    nc.sync.dma_start(out=xts[:, ki, :],
                                                      in_=xt_v[:, ki, 0:P])
                            else:
                                nc.sync.dma_start(
                                    out=xts[:], in_=xt_v[:, :, tb * P:(tb + 1) * P])
                            # cos/sin rows for this block, replicated 4x (free-dim
                            # broadcast read) so rope runs as whole-tile ops
                            ca4 = p1sb.tile([P, NH, 128], F32, tag="ca4")
                            cb4 = p1sb.tile([P, NH, 128], F32, tag="cb4")
                            nc.sync.dma_start(
                                out=ca4[:],
                                in_=csa.ap()[tb * P:(tb + 1) * P, :].unsqueeze(1)
                                .broadcast_to([P, NH, 128]))
                            nc.sync.dma_start(
                                out=cb4[:],
                                in_=csb.ap()[tb * P:(tb + 1) * P, :].unsqueeze(1)
                                .broadcast_to([P, NH, 128]))

                            q_ps = p1q.tile([P, NH * D], F32)
                            kv_ps = p1kv.tile([P, 2 * D], F32)
                            for ki in range(KT):
                                nc.tensor.matmul(q_ps[:], xts[:, ki, :], wq_sb[:, ki, :],
                                                 start=(ki == 0), stop=(ki == KT - 1))
                            for ki in range(KT):
                                nc.tensor.matmul(kv_ps[:], xts[:, ki, :], wkv_sb[:, ki, :],
                                                 start=(ki == 0), stop=(ki == KT - 1))

                            # v pass-through
                            nc.vector.tensor_copy(out=v_sb[:, tb, :], in_=kv_ps[:, D:2 * D])

                            # rope: csa = [cos|sin], csb = [sin|cos]
                            q4 = q_ps[:].rearrange("p (h d) -> p h d", h=NH)
                            pa = p1sb.tile([P, NH, D], F32, tag="pa")
                            pb = p1sb.tile([P, NH, D], F32, tag="pb")
                            pka = p1sb.tile([P, D], F32, tag="pka")
                            pkb = p1sb.tile([P, D], F32, tag="pkb")
                            nc.vector.tensor_mul(pa[:], q4, ca4[:])
                            nc.vector.tensor_mul(pb[:], q4, cb4[:])
                            nc.vector.tensor_mul(pka[:], kv_ps[:, 0:D], ca4[:, 0, :])
                            nc.vector.tensor_mul(pkb[:], kv_ps[:, 0:D], cb4[:, 0, :])

                            rq = p1sb.tile([P, NH, D], F32, tag="rq")
                            rk = p1sb.tile([P, D], F32, tag="rk")
                            nc.vector.tensor_add(rq[:, :, 0:64], pa[:, :, 0:64], pa[:, :, 64:128])
                            nc.vector.tensor_sub(rq[:, :, 64:128], pb[:, :, 64:128], pb[:, :, 0:64])
                            nc.vector.tensor_add(rk[:, 0:64], pka[:, 0:64], pka[:, 64:128])
                            nc.vector.tensor_sub(rk[:, 64:128], pkb[:, 64:128], pkb[:, 0:64])

                            # rms-norm factors (sum of squares via stt accum)
                            sqs = p1sb.tile([P, NH, D], F32, tag="sqs")
                            sqk = p1sb.tile([P, D], F32, tag="sqk")
                            ss = p1small.tile([P, 8], F32, tag="ss")
                            for h in range(NH):
                                nc.vector.scalar_tensor_tensor(
                                    out=sqs[:, h, :], in0=rq[:, h, :], scalar=1.0,
                                    in1=rq[:, h, :], op0=ALU.mult, op1=ALU.mult,
                                    accum_out=ss[:, h:h + 1])
                            nc.vector.scalar_tensor_tensor(
                                out=sqk[:], in0=rk[:], scalar=1.0,
                                in1=rk[:], op0=ALU.mult, op1=ALU.mult,
                                accum_out=ss[:, NH:NH + 1])
                            sq = p1small.tile([P, 8], F32, tag="sq")
                            nc.scalar.activation(sq[:, 0:NH + 1], ss[:, 0:NH + 1], AF.Sqrt,
                                                 bias=eps_sb[:], scale=1.0 / D)
                            rs = p1small.tile([P, 8], F32, tag="rs")
                            nc.vector.reciprocal(rs[:, 0:NH + 1], sq[:, 0:NH + 1])

                            qn = p1sb.tile([P, NH, D], F32R, tag="qn")
                            kn = p1sb.tile([P, D], F32R, tag="kn")
                            for h in range(NH):
                                nc.vector.tensor_scalar_mul(qn[:, h, :], rq[:, h, :], rs[:, h:h + 1])
                            nc.vector.tensor_scalar_mul(kn[:], rk[:], rs[:, NH:NH + 1])

                            # transposes -> [D, T1] layout
                            qt_ps = p1qt.tile([P, NH * D], F32R)
                            for h in range(NH):
                                nc.tensor.transpose(qt_ps[:, h * D:(h + 1) * D], qn[:, h, :], ident_sb[:])
                            kt_ps = p1kt.tile([P, D], F32R)
                            nc.tensor.transpose(kt_ps[:], kn[:], ident_sb[:])

                            nc.vector.tensor_copy(
                                out=qt_sb[:, :, tbl * P:(tbl + 1) * P],
                                in_=qt_ps[:].rearrange("p (h t) -> p h t", h=NH))
                            nc.vector.tensor_copy(out=kt_sb[:, tb * P:(tb + 1) * P], in_=kt_ps[:])

                    if h2 == 0:
                        # wo only needed from the first out-projection; loading
                        # here keeps it off the critical startup DMA path
                        for h in range(NH):
                            nc.sync.dma_start(out=wo_sb[:, h, :], in_=wo_v[:, h, :])

                    # ---- phase 2+3: attention (S^T layout) + out-projection ----
                    with tc.tile_pool(name="p2pt", bufs=4) as p2pt, \
                         tc.tile_pool(name="p2small", bufs=2) as p2small, \
                         tc.tile_pool(name="p3sb", bufs=3) as p3sb, \
                         tc.tile_pool(name="p2s", bufs=3, space="PSUM") as p2s, \
                         tc.tile_pool(name="p2o", bufs=2, space="PSUM") as p2o, \
                         tc.tile_pool(name="p2l", bufs=1, space="PSUM") as p2l, \
                         tc.tile_pool(name="p2rb", bufs=1, space="PSUM") as p2rb, \
                         tc.tile_pool(name="p3y", bufs=1, space="PSUM") as p3y:
                        for c2l in range(2):
                            c2 = 2 * h2 + c2l
                            for h in range(NH):
                                o_ps = p2o.tile([P, 512], F32)
                                l_ps = p2l.tile([1, 512], F32)
                                m_last = 4 * c2 + 3
                                for m in range(4 * c2 + 4):
                                    dg = m - 4 * c2
                                    n0 = 128 * dg if dg > 0 else 0
                                    w = 512 - n0
                                    s_ps = p2s.tile([P, 512], F32)
                                    nc.tensor.matmul(
                                        s_ps[:, 0:w],
                                        kt_sb[:, m * P:(m + 1) * P],
                                        qt_sb[:, h, c2l * 512 + n0: (c2l + 1) * 512],
                                        start=True, stop=True)
                                    if dg >= 0:
                                        nc.vector.tensor_add(s_ps[:, 0:P], s_ps[:, 0:P], diag_sb[:])
                                    pt = p2pt.tile([P, 512], F32R)
                                    nc.scalar.activation(pt[:, 0:w], s_ps[:, 0:w], AF.Exp,
                                                         scale=float(SCALE))
                                    nc.tensor.matmul(o_ps[:, n0:512], v_sb[:, m, :], pt[:, 0:w],
                                                     start=(m == 0), stop=(m == m_last),
                                                     skip_group_check=True)
                                    nc.tensor.matmul(l_ps[:, n0:512], onesc_sb[:], pt[:, 0:w],
                                                     start=(m == 0), stop=(m == m_last),
                                                     skip_group_check=True)

                                l_sb = p2small.tile([1, 512], F32, tag="l_sb")
                                nc.vector.tensor_copy(out=l_sb[:], in_=l_ps[:])
                                rl = p2small.tile([1, 512], F32R, tag="rl")
                                with nc.allow_low_precision(reason="f32r softmax recip for PE broadcast"):
                                    nc.vector.reciprocal(rl[:], l_sb[:])
                                rb_ps = p2rb.tile([P, 512], F32)
                                nc.tensor.matmul(rb_ps[:], onesr_sb[:], rl[:], start=True, stop=True)
                                rb_sb = p2small.tile([P, 512], F32, tag="rb_sb")
                                nc.vector.tensor_copy(out=rb_sb[:], in_=rb_ps[:])
                                nc.vector.tensor_mul(
                                    ot_sb[:, h, c2l * 512:(c2l + 1) * 512], o_ps[:], rb_sb[:])

                            # out-projection for this 512-wide T1 chunk
                            for tq in range(4):
                                tbl = c2l * 4 + tq
                                tt = h2 * (TB // 2) + tbl
                                for co in range(4):
                                    y_ps = p3y.tile([P, 512], F32)
                                    for h in range(NH):
                                        nc.tensor.matmul(
                                            y_ps[:],
                                            ot_sb[:, h, tbl * P:(tbl + 1) * P],
                                            wo_sb[:, h, co * 512:(co + 1) * 512],
                                            start=(h == 0), stop=(h == NH - 1))
                                    y_sb = p3sb.tile([P, 512], F32)
                                    nc.scalar.copy(out=y_sb[:], in_=y_ps[:])
                                    nc.sync.dma_start(
                                        out=y.ap()[tt * P:(tt + 1) * P, co * 512:(co + 1) * 512],
                                        in_=y_sb[:])

    nc.compile()
    return nc


def make_in_maps(x, cos, sin, Wq, Wk, Wv, Wo):
    """Build per-core input maps (all host-side numpy preprocessing)."""
    cos2 = np.ascontiguousarray(cos.reshape(T, 64)).astype(np.float32)
    sin2 = np.ascontiguousarray(sin.reshape(T, 64)).astype(np.float32)
    csa = np.concatenate([cos2, sin2], axis=1)          # [T, 128]
    csb = np.concatenate([sin2, cos2], axis=1)          # [T, 128]
    pp, yy = np.meshgrid(np.arange(P), np.arange(P), indexing="ij")
    diag = np.where(yy >= pp, 0.0, NEG).astype(np.float32)  # keep t1 >= t2
    ident = np.eye(P, dtype=np.float32)
    onesc = np.ones((P, 1), dtype=np.float32)
    onesr = np.ones((1, P), dtype=np.float32)

    in_maps = []
    for core in range(8):
        b, g = divmod(core, 4)
        in_maps.append({
            "xt": np.ascontiguousarray(x[b].T),
            "wq": np.ascontiguousarray(Wq[:, 512 * g:512 * (g + 1)]),
            "wkv": np.ascontiguousarray(
                np.concatenate([Wk[:, 128 * g:128 * (g + 1)],
                                Wv[:, 128 * g:128 * (g + 1)]], axis=1)),
            "wo": np.ascontiguousarray(Wo[512 * g:512 * (g + 1), :]),
            "csa": csa,
            "csb": csb,
            "diag": diag,
            "ident": ident,
            "onesc": onesc,
            "onesr": onesr,
            "epsc": np.full((P, 1), EPS, dtype=np.float32),
        })
    return in_maps


def kernel(x, cos, sin, Wq, Wk, Wv, Wo):
    global _NC_CACHE
    x = np.asarray(x, dtype=np.float32)
    cos = np.asarray(cos, dtype=np.float32)
    sin = np.asarray(sin, dtype=np.float32)
    Wq = np.asarray(Wq, dtype=np.float32)
    Wk = np.asarray(Wk, dtype=np.float32)
    Wv = np.asarray(Wv, dtype=np.float32)
    Wo = np.asarray(Wo, dtype=np.float32)

    if _NC_CACHE is None:
        _NC_CACHE = build_nc()
    nc = _NC_CACHE

    in_maps = make_in_maps(x, cos, sin, Wq, Wk, Wv, Wo)
    res = run_bass_kernel_spmd(nc, in_maps, core_ids=list(range(8)))

    out = np.zeros((2, T, C), dtype=np.float32)
    for core in range(8):
        b = core // 4
        out[b] += res.results[core]["y"]
    return out


# revision 20
# speedup vs baseline: 1.0291x; 1.0072x over previous
"""Causal GQA self-attention block (B=2, T=2048, C=2048, 16 q-heads / 4 kv-heads,
head_dim=128, RoPE + RMS-norm on q/k) for 8 Trainium2 NeuronCores.

Sharding: core = (batch b, kv-group g), b in {0,1}, g in {0..3}.
Each core computes its batch's projections for 4 q-heads + 1 kv head,
causal attention, and a partial output projection (Wo row-shard).
Host sums the 4 partials per batch.
"""

import sys

for _p in ("/opt/trn_rl_repo", "/root/.axon_site/_ro/trn_rl_repo"):
    if _p not in sys.path:
        sys.path.insert(0, _p)

import os

import numpy as np

import concourse.bass as bass  # noqa: F401
import concourse.mybir as mybir
from concourse import bacc
from concourse.tile import TileContext
from concourse.bass_utils import run_bass_kernel_spmd

P = 128
T = 2048
C = 2048
KT = C // P          # 16 contraction tiles
TB = T // P          # 16 T1 blocks
NH = 4               # q heads per core
D = 128              # head dim
SCALE = 1.0 / np.sqrt(D)
EPS = float(np.finfo(np.float32).eps)
NEG = -1.0e30

F32R = mybir.dt.float32r
F32 = mybir.dt.float32
AF = mybir.ActivationFunctionType
ALU = mybir.AluOpType

_NC_CACHE = None


def build_nc():
    nc = bacc.Bacc("TRN2", target_bir_lowering=False, debug=False)

    xt = nc.dram_tensor("xt", [C, T], F32R, kind="ExternalInput")
    wq = nc.dram_tensor("wq", [C, NH * D], F32R, kind="ExternalInput")
    wkv = nc.dram_tensor("wkv", [C, 2 * D], F32R, kind="ExternalInput")
    wo = nc.dram_tensor("wo", [NH * D, C], F32R, kind="ExternalInput")
    csa = nc.dram_tensor("csa", [T, 128], F32, kind="ExternalInput")
    csb = nc.dram_tensor("csb", [T, 128], F32, kind="ExternalInput")
    diag = nc.dram_tensor("diag", [P, P], F32, kind="ExternalInput")
    ident = nc.dram_tensor("ident", [P, P], F32R, kind="ExternalInput")
    onesc = nc.dram_tensor("onesc", [P, 1], F32R, kind="ExternalInput")
    onesr = nc.dram_tensor("onesr", [1, P], F32R, kind="ExternalInput")
    epsc = nc.dram_tensor("epsc", [P, 1], F32, kind="ExternalInput")
    y = nc.dram_tensor("y", [T, C], F32, kind="ExternalOutput")

    xt_v = xt.ap().rearrange("(co ci) t -> ci co t", ci=P)
    wq_v = wq.ap().rearrange("(co ci) n -> ci co n", ci=P)
    wkv_v = wkv.ap().rearrange("(co ci) n -> ci co n", ci=P)
    wo_v = wo.ap().rearrange("(h d) n -> d h n", d=P)

    with TileContext(nc) as tc:
        with tc.tile_pool(name="resident", bufs=1) as wpool:
            wq_sb = wpool.tile([P, KT, NH * D], F32R)
            wkv_sb = wpool.tile([P, KT, 2 * D], F32R)
            wo_sb = wpool.tile([P, NH, C], F32R)
            kt_sb = wpool.tile([P, T], F32R)
            v_sb = wpool.tile([P, TB, D], F32R)
            diag_sb = wpool.tile([P, P], F32)
            ident_sb = wpool.tile([P, P], F32R)
            onesc_sb = wpool.tile([P, 1], F32R)
            onesr_sb = wpool.tile([1, P], F32R)
            eps_sb = wpool.tile([P, 1], F32)

            # per-k-tile weight loads so the first matmul chain can start
            # as soon as its first slices land
            for ki in range(KT):
                nc.sync.dma_start(out=wq_sb[:, ki, :], in_=wq_v[:, ki, :])
            for ki in range(KT):
                nc.sync.dma_start(out=wkv_sb[:, ki, :], in_=wkv_v[:, ki, :])
            nc.sync.dma_start(out=diag_sb[:], in_=diag.ap())
            nc.sync.dma_start(out=ident_sb[:], in_=ident.ap())
            nc.sync.dma_start(out=onesc_sb[:], in_=onesc.ap())
            nc.sync.dma_start(out=onesr_sb[:], in_=onesr.ap())
            nc.sync.dma_start(out=eps_sb[:], in_=epsc.ap())

            for h2 in range(2):  # T1 halves
                with tc.tile_pool(name="halfbuf", bufs=1) as halfpool:
                    qt_sb = halfpool.tile([P, NH, T // 2], F32R)
                    ot_sb = halfpool.tile([P, NH, T // 2], F32R)

                    # ---- phase 1: projections + rope + rms + transposes ----
                    with tc.tile_pool(name="p1x", bufs=3) as p1x, \
                         tc.tile_pool(name="p1sb", bufs=2) as p1sb, \
                         tc.tile_pool(name="p1small", bufs=2) as p1small, \
                         tc.tile_pool(name="p1q", bufs=2, space="PSUM") as p1q, \
                         tc.tile_pool(name="p1kv", bufs=2, space="PSUM") as p1kv, \
                         tc.tile_pool(name="p1qt", bufs=2, space="PSUM") as p1qt, \
                         tc.tile_pool(name="p1kt", bufs=1, space="PSUM") as p1kt:
                        for tbl in range(TB // 2):
                            tb = h2 * (TB // 2) + tbl
                            xts = p1x.tile([P, KT, P], F32R, tag="xts")
                            if tb == 0:
                                # interleave with the weight loads in consumption
                                # order so the first matmul chain starts early
                                for ki in range(KT):
                                    nc.sync.dma_start(out=xts[:, ki, :],
                                                      in_=xt_v[:, ki, 0:P])
                            else:
                                nc.sync.dma_start(
                                    out=xts[:], in_=xt_v[:, :, tb * P:(tb + 1) * P])
                            # cos/sin rows for this block, replicated 4x (free-dim
                            # broadcast read) so rope runs as whole-tile ops
                            ca4 = p1sb.tile([P, NH, 128], F32, tag="ca4")
                            cb4 = p1sb.tile([P, NH, 128], F32, tag="cb4")
                            nc.sync.dma_start(
                                out=ca4[:],
                                in_=csa.ap()[tb * P:(tb + 1) * P, :].unsqueeze(1)
                                .broadcast_to([P, NH, 128]))
                            nc.sync.dma_start(
                                out=cb4[:],
                                in_=csb.ap()[tb * P:(tb + 1) * P, :].unsqueeze(1)
                                .broadcast_to([P, NH, 128]))

                            q_ps = p1q.tile([P, NH * D], F32)
                            kv_ps = p1kv.tile([P, 2 * D], F32)
                            for ki in range(KT):
                                nc.tensor.matmul(q_ps[:], xts[:, ki, :], wq_sb[:, ki, :],
                                                 start=(ki == 0), stop=(ki == KT - 1))
                            for ki in range(KT):
                                nc.tensor.matmul(kv_ps[:], xts[:, ki, :], wkv_sb[:, ki, :],
                                                 start=(ki == 0), stop=(ki == KT - 1))

                            # v pass-through
                            nc.vector.tensor_copy(out=v_sb[:, tb, :], in_=kv_ps[:, D:2 * D])

                            # rope: csa = [cos|sin], csb = [sin|cos]
                            q4 = q_ps[:].rearrange("p (h d) -> p h d", h=NH)
                            pa = p1sb.tile([P, NH, D], F32, tag="pa")
                            pb = p1sb.tile([P, NH, D], F32, tag="pb")
                            pka = p1sb.tile([P, D], F32, tag="pka")
                            pkb = p1sb.tile([P, D], F32, tag="pkb")
                            nc.vector.tensor_mul(pa[:], q4, ca4[:])
                            nc.vector.tensor_mul(pb[:], q4, cb4[:])
                            nc.vector.tensor_mul(pka[:], kv_ps[:, 0:D], ca4[:, 0, :])
                            nc.vector.tensor_mul(pkb[:], kv_ps[:, 0:D], cb4[:, 0, :])

                            rq = p1sb.tile([P, NH, D], F32, tag="rq")
                            rk = p1sb.tile([P, D], F32, tag="rk")
                            nc.vector.tensor_add(rq[:, :, 0:64], pa[:, :, 0:64], pa[:, :, 64:128])
                            nc.vector.tensor_sub(rq[:, :, 64:128], pb[:, :, 64:128], pb[:, :, 0:64])
                            nc.vector.tensor_add(rk[:, 0:64], pka[:, 0:64], pka[:, 64:128])
                            nc.vector.tensor_sub(rk[:, 64:128], pkb[:, 64:128], pkb[:, 0:64])

                            # rms-norm factors (sum of squares via stt accum)
                            sqs = p1sb.tile([P, NH, D], F32, tag="sqs")
                            sqk = p1sb.tile([P, D], F32, tag="sqk")
                            ss = p1small.tile([P, 8], F32, tag="ss")
                            for h in range(NH):
                                nc.vector.scalar_tensor_tensor(
                                    out=sqs[:, h, :], in0=rq[:, h, :], scalar=1.0,
                                    in1=rq[:, h, :], op0=ALU.mult, op1=ALU.mult,
                                    accum_out=ss[:, h:h + 1])
                            nc.vector.scalar_tensor_tensor(
                                out=sqk[:], in0=rk[:], scalar=1.0,
                                in1=rk[:], op0=ALU.mult, op1=ALU.mult,
                                accum_out=ss[:, NH:NH + 1])
                            sq = p1small.tile([P, 8], F32, tag="sq")
                            nc.scalar.activation(sq[:, 0:NH + 1], ss[:, 0:NH + 1], AF.Sqrt,
                                                 bias=eps_sb[:], scale=1.0 / D)
                            rs = p1small.tile([P, 8], F32, tag="rs")
                            nc.vector.reciprocal(rs[:, 0:NH + 1], sq[:, 0:NH + 1])

                            qn = p1sb.tile([P, NH, D], F32R, tag="qn")
                            kn = p1sb.tile([P, D], F32R, tag="kn")
                            for h in range(NH):
                                nc.vector.tensor_scalar_mul(qn[:, h, :], rq[:, h, :], rs[:, h:h + 1])
                            nc.vector.tensor_scalar_mul(kn[:], rk[:], rs[:, NH:NH + 1])

                            # transposes -> [D, T1] layout
                            qt_ps = p1qt.tile([P, NH * D], F32R)
                            for h in range(NH):
                                nc.tensor.transpose(qt_ps[:, h * D:(h + 1) * D], qn[:, h, :], ident_sb[:])
                            kt_ps = p1kt.tile([P, D], F32R)
                            nc.tensor.transpose(kt_ps[:], kn[:], ident_sb[:])

                            nc.vector.tensor_copy(
                                out=qt_sb[:, :, tbl * P:(tbl + 1) * P],
                                in_=qt_ps[:].rearrange("p (h t) -> p h t", h=NH))
                            nc.vector.tensor_copy(out=kt_sb[:, tb * P:(tb + 1) * P], in_=kt_ps[:])

                    if h2 == 0:
                        # wo only needed from the first out-projection; loading
                        # here keeps it off the critical startup DMA path
                        for h in range(NH):
                            nc.sync.dma_start(out=wo_sb[:, h, :], in_=wo_v[:, h, :])

                    # ---- phase 2+3: attention (S^T layout) + out-projection ----
                    with tc.tile_pool(name="p2pt", bufs=4) as p2pt, \
                         tc.tile_pool(name="p2small", bufs=2) as p2small, \
                         tc.tile_pool(name="p3sb", bufs=3) as p3sb, \
                         tc.tile_pool(name="p2s", bufs=3, space="PSUM") as p2s, \
                         tc.tile_pool(name="p2o", bufs=2, space="PSUM") as p2o, \
                         tc.tile_pool(name="p2l", bufs=1, space="PSUM") as p2l, \
                         tc.tile_pool(name="p2rb", bufs=1, space="PSUM") as p2rb, \
                         tc.tile_pool(name="p3y", bufs=1, space="PSUM") as p3y:
                        for c2l in range(2):
                            c2 = 2 * h2 + c2l
                            for h in range(NH):
                                o_ps = p2o.tile([P, 512], F32)
                                l_ps = p2l.tile([1, 512], F32)
                                m_last = 4 * c2 + 3
                                for m in range(4 * c2 + 4):
                                    dg = m - 4 * c2
                                    n0 = 128 * dg if dg > 0 else 0
                                    w = 512 - n0
                                    s_ps = p2s.tile([P, 512], F32)
                                    nc.tensor.matmul(
                                        s_ps[:, 0:w],
                                        kt_sb[:, m * P:(m + 1) * P],
                                        qt_sb[:, h, c2l * 512 + n0: (c2l + 1) * 512],
                                        start=True, stop=True)
                                    if dg >= 0:
                                        nc.vector.tensor_add(s_ps[:, 0:P], s_ps[:, 0:P], diag_sb[:])
                                    pt = p2pt.tile([P, 512], F32R)
                                    nc.scalar.activation(pt[:, 0:w], s_ps[:, 0:w], AF.Exp,
                                                         scale=float(SCALE))
                                    nc.tensor.matmul(o_ps[:, n0:512], v_sb[:, m, :], pt[:, 0:w],
                                                     start=(m == 0), stop=(m == m_last),
                                                     skip_group_check=True)
                                    nc.tensor.matmul(l_ps[:, n0:512], onesc_sb[:], pt[:, 0:w],
                                                     start=(m == 0), stop=(m == m_last),
                                                     skip_group_check=True)

                                l_sb = p2small.tile([1, 512], F32, tag="l_sb")
                                nc.vector.tensor_copy(out=l_sb[:], in_=l_ps[:])
                                rl = p2small.tile([1, 512], F32R, tag="rl")
                                with nc.allow_low_precision(reason="f32r softmax recip for PE broadcast"):
                                    nc.vector.reciprocal(rl[:], l_sb[:])
                                rb_ps = p2rb.tile([P, 512], F32)
                                nc.tensor.matmul(rb_ps[:], onesr_sb[:], rl[:], start=True, stop=True)
                                rb_sb = p2small.tile([P, 512], F32, tag="rb_sb")
                                nc.vector.tensor_copy(out=rb_sb[:], in_=rb_ps[:])
                                nc.vector.tensor_mul(
                                    ot_sb[:, h, c2l * 512:(c2l + 1) * 512], o_ps[:], rb_sb[:])

                            # out-projection for this 512-wide T1 chunk
                            for tq in range(4):
                                tbl = c2l * 4 + tq
                                tt = h2 * (TB // 2) + tbl
                                for co in range(4):
                                    y_ps = p3y.tile([P, 512], F32)
                                    for h in range(NH):
                                        nc.tensor.matmul(
                                            y_ps[:],
                                            ot_sb[:, h, tbl * P:(tbl + 1) * P],
                                            wo_sb[:, h, co * 512:(co + 1) * 512],
                                            start=(h == 0), stop=(h == NH - 1))
                                    y_sb = p3sb.tile([P, 512], F32)
                                    nc.vector.tensor_copy(out=y_sb[:], in_=y_ps[:])
                                    nc.sync.dma_start(
                                        out=y.ap()[tt * P:(tt + 1) * P, co * 512:(co + 1) * 512],
                                        in_=y_sb[:])

    nc.compile()
    return nc


def make_in_maps(x, cos, sin, Wq, Wk, Wv, Wo):
    """Build per-core input maps (all host-side numpy preprocessing)."""
    cos2 = np.ascontiguousarray(cos.reshape(T, 64)).astype(np.float32)
    sin2 = np.ascontiguousarray(sin.reshape(T, 64)).astype(np.float32)
    csa = np.concatenate([cos2, sin2], axis=1)          # [T, 128]
    csb = np.concatenate([sin2, cos2], axis=1)          # [T, 128]
    pp, yy = np.meshgrid(np.arange(P), np.arange(P), indexing="ij")
    diag = np.where(yy >= pp, 0.0, NEG).astype(np.float32)  # keep t1 >= t2
    ident = np.eye(P, dtype=np.float32)
    onesc = np.ones((P, 1), dtype=np.float32)
    onesr = np.ones((1, P), dtype=np.float32)

    in_maps = []
    for core in range(8):
        b, g = divmod(core, 4)
        in_maps.append({
            "xt": np.ascontiguousarray(x[b].T),
            "wq": np.ascontiguousarray(Wq[:, 512 * g:512 * (g + 1)]),
            "wkv": np.ascontiguousarray(
                np.concatenate([Wk[:, 128 * g:128 * (g + 1)],
                                Wv[:, 128 * g:128 * (g + 1)]], axis=1)),
            "wo": np.ascontiguousarray(Wo[512 * g:512 * (g + 1), :]),
            "csa": csa,
            "csb": csb,
            "diag": diag,
            "ident": ident,
            "onesc": onesc,
            "onesr": onesr,
            "epsc": np.full((P, 1), EPS, dtype=np.float32),
        })
    return in_maps


def kernel(x, cos, sin, Wq, Wk, Wv, Wo):
    global _NC_CACHE
    x = np.asarray(x, dtype=np.float32)
    cos = np.asarray(cos, dtype=np.float32)
    sin = np.asarray(sin, dtype=np.float32)
    Wq = np.asarray(Wq, dtype=np.float32)
    Wk = np.asarray(Wk, dtype=np.float32)
    Wv = np.asarray(Wv, dtype=np.float32)
    Wo = np.asarray(Wo, dtype=np.float32)

    if _NC_CACHE is None:
        _NC_CACHE = build_nc()
    nc = _NC_CACHE

    in_maps = make_in_maps(x, cos, sin, Wq, Wk, Wv, Wo)
    res = run_bass_kernel_spmd(nc, in_maps, core_ids=list(range(8)))

    out = np.zeros((2, T, C), dtype=np.float32)
    for core in range(8):
        b = core // 4
        out[b] += res.results[core]["y"]
    return out


# revision 22
# speedup vs baseline: 1.0477x; 1.0180x over previous
"""Causal GQA self-attention block (B=2, T=2048, C=2048, 16 q-heads / 4 kv-heads,
head_dim=128, RoPE + RMS-norm on q/k) for 8 Trainium2 NeuronCores.

Sharding: core = (batch b, kv-group g), b in {0,1}, g in {0..3}.
Each core computes its batch's projections for 4 q-heads + 1 kv head,
causal attention, and a partial output projection (Wo row-shard).
Host sums the 4 partials per batch.
"""

import sys

for _p in ("/opt/trn_rl_repo", "/root/.axon_site/_ro/trn_rl_repo"):
    if _p not in sys.path:
        sys.path.insert(0, _p)

import os

import numpy as np

import concourse.bass as bass  # noqa: F401
import concourse.mybir as mybir
from concourse import bacc
from concourse.tile import TileContext
from concourse.bass_utils import run_bass_kernel_spmd

P = 128
T = 2048
C = 2048
KT = C // P          # 16 contraction tiles
TB = T // P          # 16 T1 blocks
NH = 4               # q heads per core
D = 128              # head dim
SCALE = 1.0 / np.sqrt(D)
EPS = float(np.finfo(np.float32).eps)
NEG = -1.0e30

F32R = mybir.dt.float32r
F32 = mybir.dt.float32
AF = mybir.ActivationFunctionType
ALU = mybir.AluOpType

_NC_CACHE = None


def build_nc():
    nc = bacc.Bacc("TRN2", target_bir_lowering=False, debug=False)

    xt = nc.dram_tensor("xt", [C, T], F32R, kind="ExternalInput")
    wq = nc.dram_tensor("wq", [C, NH * D], F32R, kind="ExternalInput")
    wkv = nc.dram_tensor("wkv", [C, 2 * D], F32R, kind="ExternalInput")
    wo = nc.dram_tensor("wo", [NH * D, C], F32R, kind="ExternalInput")
    csa = nc.dram_tensor("csa", [T, 128], F32, kind="ExternalInput")
    csb = nc.dram_tensor("csb", [T, 128], F32, kind="ExternalInput")
    diag = nc.dram_tensor("diag", [P, P], F32, kind="ExternalInput")
    ident = nc.dram_tensor("ident", [P, P], F32R, kind="ExternalInput")
    onesc = nc.dram_tensor("onesc", [P, 1], F32R, kind="ExternalInput")
    onesr = nc.dram_tensor("onesr", [1, P], F32R, kind="ExternalInput")
    epsc = nc.dram_tensor("epsc", [P, 1], F32, kind="ExternalInput")
    y = nc.dram_tensor("y", [T, C], F32, kind="ExternalOutput")

    xt_v = xt.ap().rearrange("(co ci) t -> ci co t", ci=P)
    wq_v = wq.ap().rearrange("(co ci) n -> ci co n", ci=P)
    wkv_v = wkv.ap().rearrange("(co ci) n -> ci co n", ci=P)
    wo_v = wo.ap().rearrange("(h d) n -> d h n", d=P)

    with TileContext(nc, pool_alloc_mode="queue") as tc:
        with tc.tile_pool(name="resident", bufs=1) as wpool:
            wq_sb = wpool.tile([P, KT, NH * D], F32R)
            wkv_sb = wpool.tile([P, KT, 2 * D], F32R)
            wo_sb = wpool.tile([P, NH, C], F32R)
            kt_sb = wpool.tile([P, T], F32R)
            v_sb = wpool.tile([P, TB, D], F32R)
            diag_sb = wpool.tile([P, P], F32)
            ident_sb = wpool.tile([P, P], F32R)
            onesc_sb = wpool.tile([P, 1], F32R)
            onesr_sb = wpool.tile([1, P], F32R)
            eps_sb = wpool.tile([P, 1], F32)

            # wq/wkv are loaded interleaved with the first x block below so
            # the first projection chain starts as soon as slice 0 lands
            nc.sync.dma_start(out=diag_sb[:], in_=diag.ap())
            nc.sync.dma_start(out=ident_sb[:], in_=ident.ap())
            nc.sync.dma_start(out=onesc_sb[:], in_=onesc.ap())
            nc.sync.dma_start(out=onesr_sb[:], in_=onesr.ap())
            nc.sync.dma_start(out=eps_sb[:], in_=epsc.ap())

            for h2 in range(2):  # T1 halves
                with tc.tile_pool(name="halfbuf", bufs=1) as halfpool:
                    qt_sb = halfpool.tile([P, NH, T // 2], F32R)
                    ot_sb = halfpool.tile([P, NH, T // 2], F32R)

                    # ---- phase 1: projections + rope + rms + transposes ----
                    with tc.tile_pool(name="p1x", bufs=3) as p1x, \
                         tc.tile_pool(name="p1sb", bufs=2) as p1sb, \
                         tc.tile_pool(name="p1small", bufs=2) as p1small, \
                         tc.tile_pool(name="p1q", bufs=2, space="PSUM") as p1q, \
                         tc.tile_pool(name="p1kv", bufs=2, space="PSUM") as p1kv, \
                         tc.tile_pool(name="p1qt", bufs=2, space="PSUM") as p1qt, \
                         tc.tile_pool(name="p1kt", bufs=1, space="PSUM") as p1kt:
                        for tbl in range(TB // 2):
                            tb = h2 * (TB // 2) + tbl
                            xts = p1x.tile([P, KT, P], F32R, tag="xts")
                            if tb == 0:
                                # weight + first-x loads interleaved in exactly
                                # the order the first matmul chain consumes them
                                for ki in range(KT):
                                    nc.sync.dma_start(out=wq_sb[:, ki, :],
                                                      in_=wq_v[:, ki, :])
                                    nc.sync.dma_start(out=xts[:, ki, :],
                                                      in_=xt_v[:, ki, 0:P])
                                for ki in range(KT):
                                    nc.sync.dma_start(out=wkv_sb[:, ki, :],
                                                      in_=wkv_v[:, ki, :])
                            else:
                                nc.sync.dma_start(
                                    out=xts[:], in_=xt_v[:, :, tb * P:(tb + 1) * P])
                            # cos/sin rows for this block, replicated 4x (free-dim
                            # broadcast read) so rope runs as whole-tile ops
                            ca4 = p1sb.tile([P, NH, 128], F32, tag="ca4")
                            cb4 = p1sb.tile([P, NH, 128], F32, tag="cb4")
                            nc.sync.dma_start(
                                out=ca4[:],
                                in_=csa.ap()[tb * P:(tb + 1) * P, :].unsqueeze(1)
                                .broadcast_to([P, NH, 128]))
                            nc.sync.dma_start(
                                out=cb4[:],
                                in_=csb.ap()[tb * P:(tb + 1) * P, :].unsqueeze(1)
                                .broadcast_to([P, NH, 128]))

                            q_ps = p1q.tile([P, NH * D], F32)
                            kv_ps = p1kv.tile([P, 2 * D], F32)
                            for ki in range(KT):
                                nc.tensor.matmul(q_ps[:], xts[:, ki, :], wq_sb[:, ki, :],
                                                 start=(ki == 0), stop=(ki == KT - 1))
                            for ki in range(KT):
                                nc.tensor.matmul(kv_ps[:], xts[:, ki, :], wkv_sb[:, ki, :],
                                                 start=(ki == 0), stop=(ki == KT - 1))

                            # v pass-through
                            nc.vector.tensor_copy(out=v_sb[:, tb, :], in_=kv_ps[:, D:2 * D])

                            # rope: csa = [cos|sin], csb = [sin|cos]
                            q4 = q_ps[:].rearrange("p (h d) -> p h d", h=NH)
                            pa = p1sb.tile([P, NH, D], F32, tag="pa")
                            pb = p1sb.tile([P, NH, D], F32, tag="pb")
                            pka = p1sb.tile([P, D], F32, tag="pka")
                            pkb = p1sb.tile([P, D], F32, tag="pkb")
                            nc.vector.tensor_mul(pa[:], q4, ca4[:])
                            nc.vector.tensor_mul(pb[:], q4, cb4[:])
                            nc.vector.tensor_mul(pka[:], kv_ps[:, 0:D], ca4[:, 0, :])
                            nc.vector.tensor_mul(pkb[:], kv_ps[:, 0:D], cb4[:, 0, :])

                            rq = p1sb.tile([P, NH, D], F32, tag="rq")
                            rk = p1sb.tile([P, D], F32, tag="rk")
                            nc.vector.tensor_add(rq[:, :, 0:64], pa[:, :, 0:64], pa[:, :, 64:128])
                            nc.vector.tensor_sub(rq[:, :, 64:128], pb[:, :, 64:128], pb[:, :, 0:64])
                            nc.vector.tensor_add(rk[:, 0:64], pka[:, 0:64], pka[:, 64:128])
                            nc.vector.tensor_sub(rk[:, 64:128], pkb[:, 64:128], pkb[:, 0:64])

                            # rms-norm factors (sum of squares via stt accum)
                            sqs = p1sb.tile([P, NH, D], F32, tag="sqs")
                            sqk = p1sb.tile([P, D], F32, tag="sqk")
                            ss = p1small.tile([P, 8], F32, tag="ss")
                            for h in range(NH):
                                nc.vector.scalar_tensor_tensor(
                                    out=sqs[:, h, :], in0=rq[:, h, :], scalar=1.0,
                                    in1=rq[:, h, :], op0=ALU.mult, op1=ALU.mult,
                                    accum_out=ss[:, h:h + 1])
                            nc.vector.scalar_tensor_tensor(
                                out=sqk[:], in0=rk[:], scalar=1.0,
                                in1=rk[:], op0=ALU.mult, op1=ALU.mult,
                                accum_out=ss[:, NH:NH + 1])
                            sq = p1small.tile([P, 8], F32, tag="sq")
                            nc.scalar.activation(sq[:, 0:NH + 1], ss[:, 0:NH + 1], AF.Sqrt,
                                                 bias=eps_sb[:], scale=1.0 / D)
                            rs = p1small.tile([P, 8], F32, tag="rs")
                            nc.vector.reciprocal(rs[:, 0:NH + 1], sq[:, 0:NH + 1])

                            qn = p1sb.tile([P, NH, D], F32R, tag="qn")
                            kn = p1sb.tile([P, D], F32R, tag="kn")
                            for h in range(NH):
                                nc.vector.tensor_scalar_mul(qn[:, h, :], rq[:, h, :], rs[:, h:h + 1])
                            nc.vector.tensor_scalar_mul(kn[:], rk[:], rs[:, NH:NH + 1])

                            # transposes -> [D, T1] layout
                            qt_ps = p1qt.tile([P, NH * D], F32R)
                            for h in range(NH):
                                nc.tensor.transpose(qt_ps[:, h * D:(h + 1) * D], qn[:, h, :], ident_sb[:])
                            kt_ps = p1kt.tile([P, D], F32R)
                            nc.tensor.transpose(kt_ps[:], kn[:], ident_sb[:])

                            nc.vector.tensor_copy(
                                out=qt_sb[:, :, tbl * P:(tbl + 1) * P],
                                in_=qt_ps[:].rearrange("p (h t) -> p h t", h=NH))
                            nc.vector.tensor_copy(out=kt_sb[:, tb * P:(tb + 1) * P], in_=kt_ps[:])

                    if h2 == 0:
                        # wo only needed from the first out-projection; loading
                        # here keeps it off the critical startup DMA path
                        for h in range(NH):
                            nc.sync.dma_start(out=wo_sb[:, h, :], in_=wo_v[:, h, :])

                    # ---- phase 2+3: attention (S^T layout) + out-projection ----
                    with tc.tile_pool(name="p2pt", bufs=4) as p2pt, \
                         tc.tile_pool(name="p2small", bufs=2) as p2small, \
                         tc.tile_pool(name="p3sb", bufs=3) as p3sb, \
                         tc.tile_pool(name="p2s", bufs=3, space="PSUM") as p2s, \
                         tc.tile_pool(name="p2o", bufs=2, space="PSUM") as p2o, \
                         tc.tile_pool(name="p2l", bufs=1, space="PSUM") as p2l, \
                         tc.tile_pool(name="p2rb", bufs=1, space="PSUM") as p2rb, \
                         tc.tile_pool(name="p3y", bufs=1, space="PSUM") as p3y:
                        for c2l in range(2):
                            c2 = 2 * h2 + c2l
                            for h in range(NH):
                                o_ps = p2o.tile([P, 512], F32)
                                l_ps = p2l.tile([1, 512], F32)
                                m_last = 4 * c2 + 3
                                for m in range(4 * c2 + 4):
                                    dg = m - 4 * c2
                                    n0 = 128 * dg if dg > 0 else 0
                                    w = 512 - n0
                                    s_ps = p2s.tile([P, 512], F32)
                                    nc.tensor.matmul(
                                        s_ps[:, 0:w],
                                        kt_sb[:, m * P:(m + 1) * P],
                                        qt_sb[:, h, c2l * 512 + n0: (c2l + 1) * 512],
                                        start=True, stop=True)
                                    if dg >= 0:
                                        nc.vector.tensor_add(s_ps[:, 0:P], s_ps[:, 0:P], diag_sb[:])
                                    pt = p2pt.tile([P, 512], F32R)
                                    nc.scalar.activation(pt[:, 0:w], s_ps[:, 0:w], AF.Exp,
                                                         scale=float(SCALE))
                                    nc.tensor.matmul(o_ps[:, n0:512], v_sb[:, m, :], pt[:, 0:w],
                                                     start=(m == 0), stop=(m == m_last),
                                                     skip_group_check=True)
                                    nc.tensor.matmul(l_ps[:, n0:512], onesc_sb[:], pt[:, 0:w],
                                                     start=(m == 0), stop=(m == m_last),
                                                     skip_group_check=True)

                                l_sb = p2small.tile([1, 512], F32, tag="l_sb")
                                nc.vector.tensor_copy(out=l_sb[:], in_=l_ps[:])
                                rl = p2small.tile([1, 512], F32R, tag="rl")
                                with nc.allow_low_precision(reason="f32r softmax recip for PE broadcast"):
                                    nc.vector.reciprocal(rl[:], l_sb[:])
                                rb_ps = p2rb.tile([P, 512], F32)
                                nc.tensor.matmul(rb_ps[:], onesr_sb[:], rl[:], start=True, stop=True)
                                rb_sb = p2small.tile([P, 512], F32, tag="rb_sb")
                                nc.vector.tensor_copy(out=rb_sb[:], in_=rb_ps[:])
                                nc.vector.tensor_mul(
                                    ot_sb[:, h, c2l * 512:(c2l + 1) * 512], o_ps[:], rb_sb[:])

                            # out-projection for this 512-wide T1 chunk
                            for tq in range(4):
                                tbl = c2l * 4 + tq
                                tt = h2 * (TB // 2) + tbl
                                for co in range(4):
                                    y_ps = p3y.tile([P, 512], F32)
                                    for h in range(NH):
                                        nc.tensor.matmul(
                                            y_ps[:],
                                            ot_sb[:, h, tbl * P:(tbl + 1) * P],
                                            wo_sb[:, h, co * 512:(co + 1) * 512],
                                            start=(h == 0), stop=(h == NH - 1))
                                    y_sb = p3sb.tile([P, 512], F32)
                                    nc.vector.tensor_copy(out=y_sb[:], in_=y_ps[:])
                                    nc.sync.dma_start(
                                        out=y.ap()[tt * P:(tt + 1) * P, co * 512:(co + 1) * 512],
                                        in_=y_sb[:])

    nc.compile()
    return nc


def make_in_maps(x, cos, sin, Wq, Wk, Wv, Wo):
    """Build per-core input maps (all host-side numpy preprocessing)."""
    cos2 = np.ascontiguousarray(cos.reshape(T, 64)).astype(np.float32)
    sin2 = np.ascontiguousarray(sin.reshape(T, 64)).astype(np.float32)
    csa = np.concatenate([cos2, sin2], axis=1)          # [T, 128]
    csb = np.concatenate([sin2, cos2], axis=1)          # [T, 128]
    pp, yy = np.meshgrid(np.arange(P), np.arange(P), indexing="ij")
    diag = np.where(yy >= pp, 0.0, NEG).astype(np.float32)  # keep t1 >= t2
    ident = np.eye(P, dtype=np.float32)
    onesc = np.ones((P, 1), dtype=np.float32)
    onesr = np.ones((1, P), dtype=np.float32)

    in_maps = []
    for core in range(8):
        b, g = divmod(core, 4)
        in_maps.append({
            "xt": np.ascontiguousarray(x[b].T),
            "wq": np.ascontiguousarray(Wq[:, 512 * g:512 * (g + 1)]),
            "wkv": np.ascontiguousarray(
                np.concatenate([Wk[:, 128 * g:128 * (g + 1)],
                                Wv[:, 128 * g:128 * (g + 1)]], axis=1)),
            "wo": np.ascontiguousarray(Wo[512 * g:512 * (g + 1), :]),
            "csa": csa,
            "csb": csb,
            "diag": diag,
            "ident": ident,
            "onesc": onesc,
            "onesr": onesr,
            "epsc": np.full((P, 1), EPS, dtype=np.float32),
        })
    return in_maps


def kernel(x, cos, sin, Wq, Wk, Wv, Wo):
    global _NC_CACHE
    x = np.asarray(x, dtype=np.float32)
    cos = np.asarray(cos, dtype=np.float32)
    sin = np.asarray(sin, dtype=np.float32)
    Wq = np.asarray(Wq, dtype=np.float32)
    Wk = np.asarray(Wk, dtype=np.float32)
    Wv = np.asarray(Wv, dtype=np.float32)
    Wo = np.asarray(Wo, dtype=np.float32)

    if _NC_CACHE is None:
        _NC_CACHE = build_nc()
    nc = _NC_CACHE

    in_maps = make_in_maps(x, cos, sin, Wq, Wk, Wv, Wo)
    res = run_bass_kernel_spmd(nc, in_maps, core_ids=list(range(8)))

    out = np.zeros((2, T, C), dtype=np.float32)
    for core in range(8):
        b = core // 4
        out[b] += res.results[core]["y"]
    return out
